# revision 1
# baseline (speedup 1.0000x reference)
"""Cosine-similarity retrieval kernel for 8 Trainium2 NeuronCores.

Computes out[n, m] = <x1[n]/||x1[n]||, x2[m]/||x2[m]||> / TEMP for
x1, x2 of shape (8192, 1024) fp32 (output (8192, 8192) fp32).

Sharding: x1 rows data-parallel across the 8 cores (1024-row slabs),
x2 replicated. Each core computes its (1024, 8192) slab of the score
matrix.

Device pipeline (per core), all arithmetic on-device:
  - inputs are uploaded d-major (host transpose only, no host math):
    x1t [d, n_slab], x2t [d, m] fp32; SWDGE DMA casts f32->bf16 on the
    way into SBUF
  - row norms of the bf16 data via ones-matmul column sums of
    ACT-squared tiles (the [128,128] ones stationary operand replicates
    the column sums across all partitions for free), then Sqrt (ACT) +
    reciprocal_approx_fast (DVE)
  - x1 is pre-scaled by (1/TEMP)/||x1|| into bf16 x1n (keeps the PE
    busy from the very start of the kernel via its norm matmuls)
  - x2's 1/||x2|| column scale is applied to the PSUM result during the
    PSUM->SBUF drain (DVE tensor_mul)
  - main GEMM: bf16 matmuls, k-accumulated in PSUM, N=512 chunks
"""

import sys

if "/opt/trn_rl_repo" not in sys.path:
    sys.path.insert(0, "/opt/trn_rl_repo")

import numpy as np

TEMP = 0.05
N_CORES = 8

_CACHE = {}


def _ceil_div(a, b):
    return (a + b - 1) // b


def _build(n_slab, m, d):
    """Build + compile the per-core Bass kernel. Shapes are per-core."""
    from contextlib import ExitStack

    import concourse.mybir as mybir
    import concourse.tile as tile
    from concourse import bacc

    f32 = mybir.dt.float32
    bf16 = mybir.dt.bfloat16
    AF = mybir.ActivationFunctionType

    assert d % 128 == 0 and n_slab % 128 == 0 and m % 1024 == 0
    KT = d // 128          # contraction k-tiles
    NMT = n_slab // 128    # output row tiles
    CB = 1024              # x2 column block processed per stage-B step
    NCB = m // CB
    CHW = 512              # psum chunk width (one PSUM bank)
    a_chunks = [(i * CHW, min(CHW, n_slab - i * CHW)) for i in range(_ceil_div(n_slab, CHW))]

    nc = bacc.Bacc("TRN2", target_bir_lowering=False, debug=False,
                   num_devices=N_CORES)
    x1t = nc.declare_dram_parameter("x1t", [d, n_slab], f32, isOutput=False)
    x2t = nc.declare_dram_parameter("x2t", [d, m], f32, isOutput=False)
    out = nc.declare_dram_parameter("out", [n_slab, m], f32, isOutput=True)

    x1t_k = x1t.ap().rearrange("(kk p) n -> kk p n", p=128)
    x2t_k = x2t.ap().rearrange("(kk p) mm -> kk p mm", p=128)
    out_ap = out.ap()

    with tile.TileContext(nc) as tc, ExitStack() as ctx:
        resid = ctx.enter_context(tc.tile_pool(name="resid", bufs=1))
        x1n = resid.tile([128, KT, n_slab], bf16)   # bf16 cast of x1t
        srep2 = resid.tile([128, m], f32)           # 1/||x2|| replicated
        n1i = resid.tile([128, NMT], f32)           # (1/TEMP)/||x1|| per-partition
        ones = resid.tile([128, 128], bf16)
        nc.vector.memset(ones, 1.0)

        # 4 banks for norm accumulation (A + head-cb0 + steady-state), 4 GEMM
        normp = ctx.enter_context(tc.tile_pool(name="normp", bufs=1, space="PSUM"))
        x2p = ctx.enter_context(tc.tile_pool(name="x2p", bufs=5))
        vec = ctx.enter_context(tc.tile_pool(name="vec", bufs=2))

        # preload both ACT table sets (Square, Sqrt) off the critical path
        dum = vec.tile([128, 1], f32, tag="dum", name="dum", bufs=1)
        nc.vector.memset(dum, 1.0)
        dumo = vec.tile([128, 1], f32, tag="dumo", name="dumo", bufs=1)
        nc.scalar.activation(dumo[:], dum[:], AF.Square)
        nc.scalar.activation(dumo[:], dum[:], AF.Sqrt)

        # HAM warm-up: ~8us of dummy matmuls while the first DMAs stream in,
        # so the real matmuls start at the unthrottled 2.4 GHz clock
        wsrc = vec.tile([128, 512], bf16, tag="wsrc", name="wsrc", bufs=1)
        nc.vector.memset(wsrc, 0.0)

        def b_norm_tail(npsb, cb):
            for i in range(CB // CHW):
                off = cb * CB + i * CHW
                tmp = vec.tile([128, CHW], f32, tag="vtmp", name="b_tmp")
                nc.scalar.activation(tmp[:], npsb[i][:], AF.Sqrt)
                nc.vector.reciprocal_approx_fast(out=srep2[:, off:off + CHW],
                                                 in_=tmp[:])

        # ---- head: x1 prep interleaved with cb0's loads + norms ----
        # x1 row norms via N=1 matmuls with the squared k-tile as the
        # stationary operand: out[128,1] = sq1[:, mt-slice].T @ ones[:,0:1]
        # gives per-partition sums matching the output tiles' row axis.
        # All 8*KT matmuls accumulate into one PSUM bank: only the very
        # first carries start=True (bank-wide pending-zero), the rest
        # accumulate per-element.
        with tc.tile_pool(name="a_sq", bufs=2) as a_sq, \
             tc.tile_pool(name="b_sq0", bufs=2) as b_sq0:
            np_n1 = normp.tile([128, NMT], f32, tag="np_n1", name="np_n1")
            nps0 = [normp.tile([128, CHW], f32, tag=f"np{i}", name=f"nps0_{i}")
                    for i in range(CB // CHW)]
            for _ in range(16):
                nc.tensor.matmul(nps0[0][:], ones[:, :128], wsrc[:],
                                 start=True, stop=True)
            x2cb0 = x2p.tile([128, KT, CB], bf16, tag="x2cb", name="x2cb0")
            for k in range(KT):
                # SWDGE DMAs with inline f32->bf16 cast, x1/x2 interleaved
                nc.gpsimd.dma_start(out=x1n[:, k, :], in_=x1t_k[k])
                nc.gpsimd.dma_start(out=x2cb0[:, k, :], in_=x2t_k[k][:, 0:CB])
                sq = a_sq.tile([128, n_slab], bf16, tag="a_sq", name="a_sqt")
                nc.scalar.activation(sq[:], x1n[:, k, :], AF.Square)
                for mt in range(NMT):
                    nc.tensor.matmul(np_n1[:, mt:mt + 1],
                                     sq[:, mt * 128:(mt + 1) * 128],
                                     ones[:, 0:1],
                                     start=(k == 0 and mt == 0),
                                     stop=(k == KT - 1 and mt == NMT - 1),
                                     skip_group_check=True)
                sqb = b_sq0.tile([128, CB], bf16, tag="b_sq0", name="b_sqt0")
                nc.scalar.activation(sqb[:], x2cb0[:, k, :], AF.Square)
                for i in range(CB // CHW):
                    nc.tensor.matmul(nps0[i][:], ones[:, :128],
                                     sqb[:, i * CHW:(i + 1) * CHW],
                                     start=(k == 0), stop=(k == KT - 1))
            tmp8 = vec.tile([128, NMT], f32, tag="tmp8", name="tmp8", bufs=1)
            # sqrt(nsq * TEMP^2) = ||x1|| * TEMP ; reciprocal -> (1/TEMP)/||x1||
            nc.scalar.activation(tmp8[:], np_n1[:], AF.Sqrt,
                                 scale=float(TEMP * TEMP))
            nc.vector.reciprocal_approx_fast(out=n1i[:], in_=tmp8[:])
            b_norm_tail(nps0, 0)

        # ------------- stages B+C interleaved over column blocks -------------
        with tc.tile_pool(name="b_sq", bufs=2) as b_sq, \
             tc.tile_pool(name="cps", bufs=2, space="PSUM") as cps, \
             tc.tile_pool(name="ost", bufs=3) as ost:
            for cb in range(NCB):
                csl = slice(cb * CB, (cb + 1) * CB)
                if cb > 0:
                    # -- stage B: cast-DMA + norms for this column block
                    x2cb = x2p.tile([128, KT, CB], bf16, tag="x2cb", name="x2cb")
                    npsb = [normp.tile([128, CHW], f32, tag=f"np{i}",
                                       name=f"npsB{i}")
                            for i in range(CB // CHW)]
                    for k in range(KT):
                        nc.gpsimd.dma_start(out=x2cb[:, k, :],
                                            in_=x2t_k[k][:, csl])
                        sq = b_sq.tile([128, CB], bf16, tag="b_sq", name="b_sqt")
                        nc.scalar.activation(sq[:], x2cb[:, k, :], AF.Square)
                        for i in range(CB // CHW):
                            nc.tensor.matmul(npsb[i][:], ones[:, :128],
                                             sq[:, i * CHW:(i + 1) * CHW],
                                             start=(k == 0), stop=(k == KT - 1))
                    b_norm_tail(npsb, cb)
                else:
                    x2cb = x2cb0
                # -- stage C: output tiles of this column block
                for mt in range(NMT):
                    ps = cps.tile([128, CB], f32, tag="c_ps", name="c_ps")
                    for i in range(CB // CHW):
                        for k in range(KT):
                            nc.tensor.matmul(
                                ps[:, i * CHW:(i + 1) * CHW],
                                x1n[:, k, mt * 128:(mt + 1) * 128],
                                x2cb[:, k, i * CHW:(i + 1) * CHW],
                                start=(k == 0), stop=(k == KT - 1))
                    ot = ost.tile([128, CB], f32, tag="c_ot", name="c_ot")
                    # out = (psum * (1/TEMP)/||x1||_row) * (1/||x2||)_col
                    nc.vector.scalar_tensor_tensor(
                        out=ot[:], in0=ps[:], scalar=n1i[:, mt:mt + 1],
                        in1=srep2[:, csl],
                        op0=mybir.AluOpType.mult, op1=mybir.AluOpType.mult)
                    nc.sync.dma_start(
                        out=out_ap[mt * 128:(mt + 1) * 128, csl], in_=ot[:])

    nc.compile()
    return nc


def _get_nc(n_slab, m, d):
    key = (n_slab, m, d)
    if key not in _CACHE:
        _CACHE[key] = _build(n_slab, m, d)
    return _CACHE[key]


def _in_maps(x1, x2, n_slab):
    x1t = np.ascontiguousarray(x1.T)  # [d, n]
    x2t = np.ascontiguousarray(x2.T)  # [d, m]
    return [
        {"x1t": np.ascontiguousarray(x1t[:, i * n_slab:(i + 1) * n_slab]),
         "x2t": x2t}
        for i in range(N_CORES)
    ]


def kernel(x1, x2):
    from concourse.bass_utils import run_bass_kernel_spmd

    x1 = np.asarray(x1, dtype=np.float32)
    x2 = np.asarray(x2, dtype=np.float32)
    n, d = x1.shape
    m, d2 = x2.shape
    assert d == d2 and n % N_CORES == 0
    n_slab = n // N_CORES

    nc = _get_nc(n_slab, m, d)
    res = run_bass_kernel_spmd(nc, _in_maps(x1, x2, n_slab),
                               core_ids=list(range(N_CORES)))
    return np.concatenate([res.results[i]["out"] for i in range(N_CORES)], axis=0)


if __name__ == "__main__":
    # small-shape self test
    rng = np.random.default_rng(0)
    n, m, d = 1024, 2048, 256
    x1 = rng.standard_normal((n, d), dtype=np.float32)
    x2 = rng.standard_normal((m, d), dtype=np.float32)
    got = kernel(x1, x2)
    x1n = x1 / np.linalg.norm(x1, axis=1, keepdims=True)
    x2n = x2 / np.linalg.norm(x2, axis=1, keepdims=True)
    want = (x1n @ x2n.T) / TEMP
    rel = np.linalg.norm(got - want) / np.linalg.norm(want)
    print("rel l2 err:", rel)
    print("max abs err:", np.abs(got - want).max(), "scale:", np.abs(want).max())



# revision 3
# speedup vs baseline: 1.0476x; 1.0476x over previous
"""Cosine-similarity retrieval kernel for 8 Trainium2 NeuronCores.

Computes out[n, m] = <x1[n]/||x1[n]||, x2[m]/||x2[m]||> / TEMP for
x1, x2 of shape (8192, 1024) fp32 (output (8192, 8192) fp32).

Sharding: x1 rows data-parallel across the 8 cores (1024-row slabs),
x2 replicated. Each core computes its (1024, 8192) slab of the score
matrix.

Device pipeline (per core), all arithmetic on-device:
  - inputs are uploaded d-major (host transpose only, no host math):
    x1t [d, n_slab], x2t [d, m] fp32; SWDGE DMA casts f32->bf16 on the
    way into SBUF
  - x2 row norms via fp8(e4m3) squares + DoubleRow ones-matmul column
    sums (2 k-tiles per pass, 0.5 cyc/row: 4x cheaper on the PE than
    the bf16 variant), then Sqrt (ACT) + reciprocal_approx_fast (DVE)
  - x1 row norms via N=1 matmuls with the squared k-tile stationary
    (gives the per-partition layout the drain needs directly)
  - head is ordered x1 -> x2[cb0 chunk0] -> x2[cb0 chunk1] so the cb0
    GEMM (chunk-major) starts as soon as x1 + 2MB of x2 have landed;
    dummy matmuls keep the PE busy through the DMA window so the HAM
    clock ramps to max early and never drops
  - main GEMM: bf16 matmuls, k-accumulated in PSUM, 512-wide chunks
    (one PSUM bank each), drained per-chunk by a DVE
    scalar_tensor_tensor that applies both norm scales
"""

import sys

if "/opt/trn_rl_repo" not in sys.path:
    sys.path.insert(0, "/opt/trn_rl_repo")

import numpy as np

TEMP = 0.05
N_CORES = 8

_CACHE = {}


def _build(n_slab, m, d):
    """Build + compile the per-core Bass kernel. Shapes are per-core."""
    from contextlib import ExitStack

    import concourse.mybir as mybir
    import concourse.tile as tile
    from concourse import bacc

    f32 = mybir.dt.float32
    bf16 = mybir.dt.bfloat16
    f8e4 = mybir.dt.float8e4
    AF = mybir.ActivationFunctionType
    DR = mybir.MatmulPerfMode.DoubleRow

    assert d % 256 == 0 and n_slab % 128 == 0 and m % 1024 == 0
    KT = d // 128          # contraction k-tiles
    NMT = n_slab // 128    # output row tiles
    CB = 1024              # x2 column block per stage-B step
    NCB = m // CB
    CHW = 512              # psum chunk width (one PSUM bank)
    NCHK = CB // CHW

    nc = bacc.Bacc("TRN2", target_bir_lowering=False, debug=False,
                   num_devices=N_CORES)
    x1t = nc.declare_dram_parameter("x1t", [d, n_slab], f32, isOutput=False)
    x2t = nc.declare_dram_parameter("x2t", [d, m], f32, isOutput=False)
    out = nc.declare_dram_parameter("out", [n_slab, m], f32, isOutput=True)

    x1t_k = x1t.ap().rearrange("(kk p) n -> kk p n", p=128)
    x2t_k = x2t.ap().rearrange("(kk p) mm -> kk p mm", p=128)
    out_ap = out.ap()

    with tile.TileContext(nc) as tc, ExitStack() as ctx:
        resid = ctx.enter_context(tc.tile_pool(name="resid", bufs=1))
        x1n = resid.tile([128, KT, n_slab], bf16)   # bf16 cast of x1t
        srep2 = resid.tile([128, m], f32)           # 1/||x2|| replicated
        n1i = resid.tile([128, NMT], f32)           # (1/TEMP)/||x1|| per-part
        ones = resid.tile([128, 128], bf16)
        ones8 = resid.tile([128, 2, 128], f8e4)     # DoubleRow ones stationary
        wsrc = resid.tile([128, CHW], bf16)         # dummy-fill moving operand
        nc.vector.memset(ones, 1.0)
        nc.vector.memset(ones8, 1.0)
        nc.vector.memset(wsrc, 0.0)

        # PSUM: np_n1 (1 bank) + np0/np1 (2) + dummy (1) + cps (4) = 8
        normp = ctx.enter_context(tc.tile_pool(name="normp", bufs=1,
                                               space="PSUM"))
        np_n1 = normp.tile([128, NMT], f32, tag="np_n1", name="np_n1")
        dum_ps = normp.tile([128, CHW], f32, tag="dum_ps", name="dum_ps")

        x2p = ctx.enter_context(tc.tile_pool(name="x2p", bufs=5))
        sq8p = ctx.enter_context(tc.tile_pool(name="sq8p", bufs=2))
        vec = ctx.enter_context(tc.tile_pool(name="vec", bufs=2))

        # preload both ACT table sets (Square, Sqrt) off the critical path
        dum = vec.tile([128, 1], f32, tag="dum", name="dum", bufs=1)
        nc.vector.memset(dum, 1.0)
        dumo = vec.tile([128, 1], f32, tag="dumo", name="dumo", bufs=1)
        nc.scalar.activation(dumo[:], dum[:], AF.Square)
        nc.scalar.activation(dumo[:], dum[:], AF.Sqrt)

        def fill(nmm):
            # HAM filler: keeps the PE streaming while DMAs land so the
            # clock ramps to max early and stays there
            for _ in range(nmm):
                nc.tensor.matmul(dum_ps[:], ones[:, :128], wsrc[:],
                                 start=True, stop=True, skip_group_check=True)

        def norm_chunk(sq8t, cb, c):
            # column sums of fp8 squares for one 512-col chunk via
            # DoubleRow ones-matmuls (2 k-tiles per pass), then
            # sqrt + reciprocal into the replicated srep2 row
            npsb = normp.tile([128, CHW], f32, tag=f"np{c}", name=f"np{c}")
            for j in range(KT // 2):
                nc.tensor.matmul(npsb[:], ones8[:, :, :],
                                 sq8t[:, 2 * j:2 * j + 2,
                                      c * CHW:(c + 1) * CHW],
                                 start=(j == 0), stop=(j == KT // 2 - 1),
                                 perf_mode=DR)
            tmp = vec.tile([128, CHW], f32, tag="vtmp", name="b_tmp")
            nc.scalar.activation(tmp[:], npsb[:], AF.Sqrt)
            off = cb * CB + c * CHW
            nc.vector.reciprocal_approx_fast(out=srep2[:, off:off + CHW],
                                             in_=tmp[:])

        cps = ctx.enter_context(tc.tile_pool(name="cps", bufs=4,
                                             space="PSUM"))
        ost = ctx.enter_context(tc.tile_pool(name="ost", bufs=4))

        def gemm_chunk(x2cb, cb, mt, c):
            ps = cps.tile([128, CHW], f32, tag="c_ps", name="c_ps")
            for k in range(KT):
                nc.tensor.matmul(ps[:],
                                 x1n[:, k, mt * 128:(mt + 1) * 128],
                                 x2cb[:, k, c * CHW:(c + 1) * CHW],
                                 start=(k == 0), stop=(k == KT - 1))
            return ps

        def drain(ps, cb, mt, c):
            csl = slice(cb * CB + c * CHW, cb * CB + (c + 1) * CHW)
            ot = ost.tile([128, CHW], f32, tag="c_ot", name="c_ot")
            # out = (psum * (1/TEMP)/||x1||_row) * (1/||x2||)_col
            nc.vector.scalar_tensor_tensor(
                out=ot[:], in0=ps[:], scalar=n1i[:, mt:mt + 1],
                in1=srep2[:, csl],
                op0=mybir.AluOpType.mult, op1=mybir.AluOpType.mult)
            nc.sync.dma_start(out=out_ap[mt * 128:(mt + 1) * 128, csl],
                              in_=ot[:])

        # ---- head: x1 loads + norms, then cb0 chunk loads ----
        with tc.tile_pool(name="a_sq", bufs=2) as a_sq:
            fill(10)
            for k in range(KT):
                # SWDGE DMAs with inline f32->bf16 cast
                nc.gpsimd.dma_start(out=x1n[:, k, :], in_=x1t_k[k])
                sq = a_sq.tile([128, n_slab], bf16, tag="a_sq", name="a_sqt")
                nc.scalar.activation(sq[:], x1n[:, k, :], AF.Square)
                # x1 row norms: N=1 matmuls, squared k-tile stationary;
                # all accumulate into one PSUM bank (single start/stop)
                for mt in range(NMT):
                    nc.tensor.matmul(np_n1[:, mt:mt + 1],
                                     sq[:, mt * 128:(mt + 1) * 128],
                                     ones[:, 0:1],
                                     start=(k == 0 and mt == 0),
                                     stop=(k == KT - 1 and mt == NMT - 1),
                                     skip_group_check=True)
                fill(3)
            x2cb0 = x2p.tile([128, KT, CB], bf16, tag="x2cb", name="x2cb0")
            sq8_0 = sq8p.tile([128, KT, CB], f8e4, tag="sq8", name="sq8_0")
            for c in range(NCHK):
                for k in range(KT):
                    cs = slice(c * CHW, (c + 1) * CHW)
                    nc.gpsimd.dma_start(out=x2cb0[:, k, cs],
                                        in_=x2t_k[k][:, cs])
                    nc.scalar.activation(sq8_0[:, k, cs], x2cb0[:, k, cs],
                                         AF.Square)
                    if c == 0:
                        fill(1)
            tmp8 = vec.tile([128, NMT], f32, tag="tmp8", name="tmp8", bufs=1)
            # sqrt(nsq * TEMP^2) = ||x1||*TEMP ; reciprocal -> (1/TEMP)/||x1||
            nc.scalar.activation(tmp8[:], np_n1[:], AF.Sqrt,
                                 scale=float(TEMP * TEMP))
            nc.vector.reciprocal_approx_fast(out=n1i[:], in_=tmp8[:])

        # ---- cb0 stage C, chunk-major so chunk0 starts on partial x2 ----
        # norm_chunk(0,0) must be issued before drain(mt0,c0): the DVE
        # queue is in-order and the STT reads srep2. Chunk1's norms go
        # after mt5 of the c0 pass (its squares land later).
        flush_mt = min(1, NMT - 1)
        norm1_mt = min(5, NMT - 1)
        for c in range(NCHK):
            pend = []
            for mt in range(NMT):
                ps = gemm_chunk(x2cb0, 0, mt, c)
                if c == 0 and mt <= flush_mt:
                    pend.append((ps, mt))
                    if mt == flush_mt:
                        norm_chunk(sq8_0, 0, 0)
                        for ps_, mt_ in pend:
                            drain(ps_, 0, mt_, 0)
                        pend = []
                else:
                    drain(ps, 0, mt, c)
                if c == 0 and mt == norm1_mt:
                    norm_chunk(sq8_0, 0, 1)

        # ------------- stages B+C over remaining column blocks -------------
        for cb in range(1, NCB):
            csl = slice(cb * CB, (cb + 1) * CB)
            x2cb = x2p.tile([128, KT, CB], bf16, tag="x2cb", name="x2cb")
            sq8t = sq8p.tile([128, KT, CB], f8e4, tag="sq8", name="sq8t")
            for k in range(KT):
                nc.gpsimd.dma_start(out=x2cb[:, k, :], in_=x2t_k[k][:, csl])
                nc.scalar.activation(sq8t[:, k, :], x2cb[:, k, :], AF.Square)
            norm_chunk(sq8t, cb, 0)
            norm_chunk(sq8t, cb, 1)
            for mt in range(NMT):
                for c in range(NCHK):
                    ps = gemm_chunk(x2cb, cb, mt, c)
                    drain(ps, cb, mt, c)

    nc.compile()
    return nc


def _get_nc(n_slab, m, d):
    key = (n_slab, m, d)
    if key not in _CACHE:
        _CACHE[key] = _build(n_slab, m, d)
    return _CACHE[key]


def _in_maps(x1, x2, n_slab):
    x1t = np.ascontiguousarray(x1.T)  # [d, n]
    x2t = np.ascontiguousarray(x2.T)  # [d, m]
    return [
        {"x1t": np.ascontiguousarray(x1t[:, i * n_slab:(i + 1) * n_slab]),
         "x2t": x2t}
        for i in range(N_CORES)
    ]


def kernel(x1, x2):
    from concourse.bass_utils import run_bass_kernel_spmd

    x1 = np.asarray(x1, dtype=np.float32)
    x2 = np.asarray(x2, dtype=np.float32)
    n, d = x1.shape
    m, d2 = x2.shape
    assert d == d2 and n % N_CORES == 0
    n_slab = n // N_CORES

    nc = _get_nc(n_slab, m, d)
    res = run_bass_kernel_spmd(nc, _in_maps(x1, x2, n_slab),
                               core_ids=list(range(N_CORES)))
    return np.concatenate([res.results[i]["out"] for i in range(N_CORES)],
                          axis=0)


if __name__ == "__main__":
    # small-shape self test
    rng = np.random.default_rng(0)
    n, m, d = 1024, 2048, 256
    x1 = rng.standard_normal((n, d), dtype=np.float32)
    x2 = rng.standard_normal((m, d), dtype=np.float32)
    got = kernel(x1, x2)
    x1n = x1 / np.linalg.norm(x1, axis=1, keepdims=True)
    x2n = x2 / np.linalg.norm(x2, axis=1, keepdims=True)
    want = (x1n @ x2n.T) / TEMP
    rel = np.linalg.norm(got - want) / np.linalg.norm(want)
    print("rel l2 err:", rel)
    print("max abs err:", np.abs(got - want).max(), "scale:", np.abs(want).max())
